# revision 6
# baseline (speedup 1.0000x reference)
"""RBF Gram matrix kernel for TRN2: out[i,j] = exp(-||x_i - y_j||^2).

x, y: [8192, 64] fp32 -> out [8192, 8192] fp32.

Sharding: x rows split across 8 NeuronCores (1024 rows each), y replicated.
Each core computes a [1024, 8192] tile of the Gram matrix.

Math: s = |x|^2 + |y|^2 - 2 x.y is accumulated in PSUM by two bf16
matmuls using a hi/lo mantissa split (x = xh + xl, y = yh + yl):
  MM1 (K=128): [xh; xl]^T   @ [2yh; 2yh]          -> 2(xh+xl).yh
  MM2 (K=68):  [xh; 1; 1; xsq_h; xsq_l]^T
               @ [2yl; -ysq_h; -ysq_l; -1; -1]    -> 2 xh.yl - |y|^2 - |x|^2
(The dropped xl.yl term is ~1e-4 relative.) PSUM then holds -s, and one
ScalarE Exp pass writes exp(-s) to SBUF, which DMAs to HBM. The kernel is
memory-bound on the 32 MiB/core output write.
"""

import numpy as np
import ml_dtypes

import concourse.bass as bass
import concourse.tile as tile
import concourse.mybir as mybir
from concourse.bass_utils import run_bass_kernel_spmd

N_CORES = 8
N_ROWS = 8192          # x rows (Gram rows), sharded
N_COLS = 8192          # y rows (Gram cols), replicated
D = 64
RPC = N_ROWS // N_CORES  # 1024 rows per core

DT = mybir.dt.float32
BF = mybir.dt.bfloat16
K1 = 2 * D             # 128: [xh; xl] rows
K2 = D + 4             # 68: [xh; 1; 1; xsq_h; xsq_l] rows
W = RPC + N_COLS       # packed input width: lhsT cols then rhs cols

R_TILES = RPC // 128   # 8 row tiles of 128 partitions
CBW = 2048             # column block width (4 PSUM banks)
MM_W = 512             # one matmul free dim (1 PSUM bank, fp32)
C_BLOCKS = N_COLS // CBW


def _split_excess_waits(nc, limits=None):
    """The walrus in this container accepts only a small number of sync-wait
    commands per instruction (1 for Drain, ~2 elsewhere). Hoist excess waits
    onto injected NoOps on the same engine, placed just before the original
    instruction so per-engine ordering (and thus the waits) is preserved."""
    if limits is None:
        limits = {"InstNoOp": 1, "default": 1}
    n_split = 0
    for f in nc.m.functions:
        for blk in f.blocks:
            insts = blk.instructions
            i = 0
            while i < len(insts):
                inst = insts[i]
                si = inst.sync_info
                lim = limits.get(type(inst).__name__, limits["default"])
                if si is not None and len(si.on_wait) > lim:
                    waits = list(si.on_wait)
                    keep = waits[-lim:] if lim > 0 else []
                    excess = waits[:-lim] if lim > 0 else waits
                    per_nop = limits["InstNoOp"]
                    chunks = [
                        excess[j:j + per_nop] for j in range(0, len(excess), per_nop)
                    ]
                    for k, ch in enumerate(chunks):
                        nop = mybir.InstNoOp(
                            name=nc.get_next_instruction_name(),
                            sync_info=mybir.SyncInfo(on_wait=ch, on_update=[]),
                            bass_nofuse=True,
                            engine=inst.engine,
                        )
                        nc.register_instruction(nop)
                        insts.insert(i + k, nop)
                    si.on_wait = keep
                    i += len(chunks)
                    n_split += 1
                i += 1
    return n_split


def build_nc(loop_r=None):
    import contextlib

    nc = bass.Bass()
    p1_d = nc.dram_tensor("p1", [K1, W], BF, kind="ExternalInput")
    p2_d = nc.dram_tensor("p2", [K2, W], BF, kind="ExternalInput")
    out_d = nc.dram_tensor("out", [RPC, N_COLS], DT, kind="ExternalOutput")
    NCH = N_COLS // CBW  # rhs DMA chunk width == column block width

    with tile.TileContext(nc) as tc:
        with (
            tc.tile_pool(name="inp", bufs=1) as sbin,
            tc.tile_pool(name="outp", bufs=4) as sbout,
            tc.tile_pool(name="ps", bufs=2, space="PSUM") as ps,
        ):
            # warm the ACT exp table-set load (~2.7 us) under the input DMAs
            warm = sbout.tile([128, 8], DT, name="actwarm")
            nc.scalar.activation(warm[:], warm[:], mybir.ActivationFunctionType.Exp)

            loop = tc.For_i(0, loop_r) if loop_r else contextlib.nullcontext()
            with loop:
                # chunked input tiles: the first matmuls gate on the lhs chunks
                # plus one 2048-col rhs chunk instead of the whole 3.45 MiB input
                p1_lhs = sbin.tile([K1, RPC], BF, name="p1l")
                p2_lhs = sbin.tile([K2, RPC], BF, name="p2l")
                p1_rhs = [sbin.tile([K1, CBW], BF, name=f"p1r{h}") for h in range(NCH)]
                p2_rhs = [sbin.tile([K2, CBW], BF, name=f"p2r{h}") for h in range(NCH)]
                nc.sync.dma_start(p1_lhs[:], p1_d[:, :RPC])
                nc.sync.dma_start(p2_lhs[:], p2_d[:, :RPC])
                for h in range(NCH):
                    c0 = RPC + h * CBW
                    nc.sync.dma_start(p1_rhs[h][:], p1_d[:, c0:c0 + CBW])
                    nc.sync.dma_start(p2_rhs[h][:], p2_d[:, c0:c0 + CBW])

                n_dma = 0
                for r in range(R_TILES):
                    lhs1 = p1_lhs[:, r * 128:(r + 1) * 128]
                    lhs2 = p2_lhs[:, r * 128:(r + 1) * 128]
                    for cb in range(C_BLOCKS):
                        acc = ps.tile([128, CBW], DT)
                        for j in range(CBW // MM_W):
                            o = j * MM_W
                            seg = slice(j * MM_W, (j + 1) * MM_W)
                            nc.tensor.matmul(
                                acc[:, seg], lhs1, p1_rhs[cb][:, o:o + MM_W],
                                start=True, stop=False,
                            )
                            nc.tensor.matmul(
                                acc[:, seg], lhs2, p2_rhs[cb][:, o:o + MM_W],
                                start=False, stop=True,
                            )
                        ot = sbout.tile([128, CBW], DT)
                        nc.scalar.activation(
                            ot[:], acc[:], mybir.ActivationFunctionType.Exp
                        )
                        # alternate between the two HWDGE rings (SP and ACT)
                        eng = nc.scalar if n_dma % 2 else nc.sync
                        eng.dma_start(
                            out_d[r * 128:(r + 1) * 128, cb * CBW:(cb + 1) * CBW],
                            ot[:],
                        )
                        n_dma += 1
    _split_excess_waits(nc)
    return nc


def _bf(a):
    return a.astype(ml_dtypes.bfloat16)


def prepare_in_maps(x, y):
    x = np.asarray(x, dtype=np.float32)
    y = np.asarray(y, dtype=np.float32)
    assert x.shape == (N_ROWS, D) and y.shape == (N_COLS, D)

    x_sq = (x * x).sum(axis=1, dtype=np.float32)
    y_sq = (y * y).sum(axis=1, dtype=np.float32)

    xh = _bf(x)
    xl = _bf(x - xh.astype(np.float32))
    yh = _bf(y)
    yl2 = _bf(2.0 * (y - yh.astype(np.float32)))
    xsq_h = _bf(x_sq)
    xsq_l = _bf(x_sq - xsq_h.astype(np.float32))
    ysq_h = _bf(y_sq)
    ysq_l = _bf(y_sq - ysq_h.astype(np.float32))

    # rhs halves are shared by all cores
    rhs1 = np.concatenate([2 * yh.T, 2 * yh.T], axis=0).astype(ml_dtypes.bfloat16)
    ones_n = np.ones((1, N_COLS), ml_dtypes.bfloat16)
    rhs2 = np.concatenate(
        [yl2.T, -ysq_h[None, :], -ysq_l[None, :], -ones_n, -ones_n], axis=0
    ).astype(ml_dtypes.bfloat16)

    in_maps = []
    for c in range(N_CORES):
        rows = slice(c * RPC, (c + 1) * RPC)
        ones_m = np.ones((1, RPC), ml_dtypes.bfloat16)
        lhs1 = np.concatenate([xh.T[:, rows], xl.T[:, rows]], axis=0)
        lhs2 = np.concatenate(
            [xh.T[:, rows], ones_m, ones_m,
             xsq_h[None, rows], xsq_l[None, rows]], axis=0
        )
        p1 = np.concatenate([lhs1, rhs1], axis=1).astype(ml_dtypes.bfloat16)
        p2 = np.concatenate([lhs2, rhs2], axis=1).astype(ml_dtypes.bfloat16)
        in_maps.append({"p1": p1, "p2": p2})
    return in_maps


def kernel(x, y):
    in_maps = prepare_in_maps(x, y)
    nc = build_nc()
    res = run_bass_kernel_spmd(nc, in_maps, core_ids=list(range(N_CORES)))
    return np.concatenate([res.results[c]["out"] for c in range(N_CORES)], axis=0)



# revision 10
# speedup vs baseline: 69.9730x; 69.9730x over previous
"""RBF Gram matrix kernel for TRN2: out[i,j] = exp(-||x_i - y_j||^2).

x, y: [8192, 64] fp32 -> out [8192, 8192] fp32.

Sharding: x rows split across 8 NeuronCores (1024 rows each), y replicated.
Each core computes a [1024, 8192] tile of the Gram matrix.

Math: one K=66 float32r matmul per output tile accumulates
  PSUM = x_i . 2y_j - |x_i|^2 - |y_j|^2 = -s_ij
via packed operands  lhsT = [x.T; xsq; 1]  rhs = [2y.T; -1; -ysq]
(fp32r streams at bf16 rate for free dims >= 256, ~1.7e-4 matmul error).
One ScalarE Exp pass reads PSUM and writes bf16 to SBUF (halving the HBM
write traffic vs fp32); the host upcasts to fp32. The kernel is bound by
the ScalarE exp pass (~55 us/core) and the 16 MiB/core output DMA.
"""

import numpy as np
import ml_dtypes

import concourse.bass as bass
import concourse.tile as tile
import concourse.mybir as mybir
from concourse.bass_utils import run_bass_kernel_spmd

N_CORES = 8
N_ROWS = 8192          # x rows (Gram rows), sharded
N_COLS = 8192          # y rows (Gram cols), replicated
D = 64
RPC = N_ROWS // N_CORES  # 1024 rows per core

F32 = mybir.dt.float32
F32R = mybir.dt.float32r
BF = mybir.dt.bfloat16
K = D + 2              # 66: [x; xsq; 1] rows
W = RPC + N_COLS       # packed input width: lhsT cols then rhs cols

R_TILES = RPC // 128   # 8 row tiles of 128 partitions
CBW = 2048             # column block width (4 PSUM banks)
MM_W = 512             # one matmul free dim (1 PSUM bank, fp32)
C_BLOCKS = N_COLS // CBW


def _split_excess_waits(nc, limits=None):
    """The walrus in this container accepts only a small number of sync-wait
    commands per instruction (1 for Drain, ~2 elsewhere). Hoist excess waits
    onto injected NoOps on the same engine, placed just before the original
    instruction so per-engine ordering (and thus the waits) is preserved."""
    if limits is None:
        limits = {"InstNoOp": 1, "default": 1}
    n_split = 0
    for f in nc.m.functions:
        for blk in f.blocks:
            insts = blk.instructions
            i = 0
            while i < len(insts):
                inst = insts[i]
                si = inst.sync_info
                lim = limits.get(type(inst).__name__, limits["default"])
                if si is not None and len(si.on_wait) > lim:
                    waits = list(si.on_wait)
                    keep = waits[-lim:] if lim > 0 else []
                    excess = waits[:-lim] if lim > 0 else waits
                    per_nop = limits["InstNoOp"]
                    chunks = [
                        excess[j:j + per_nop] for j in range(0, len(excess), per_nop)
                    ]
                    for k, ch in enumerate(chunks):
                        nop = mybir.InstNoOp(
                            name=nc.get_next_instruction_name(),
                            sync_info=mybir.SyncInfo(on_wait=ch, on_update=[]),
                            bass_nofuse=True,
                            engine=inst.engine,
                        )
                        nc.register_instruction(nop)
                        insts.insert(i + k, nop)
                    si.on_wait = keep
                    i += len(chunks)
                    n_split += 1
                i += 1
    return n_split


def build_nc(loop_r=None):
    import contextlib

    nc = bass.Bass()
    p_d = nc.dram_tensor("p", [K, W], F32R, kind="ExternalInput")
    out_d = nc.dram_tensor("out", [RPC, N_COLS], BF, kind="ExternalOutput")
    tick_d = None
    if loop_r:
        tick_d = nc.dram_tensor("tick", [128, 8], F32, kind="ExternalOutput")
    NCH = N_COLS // CBW  # rhs DMA chunk width == column block width

    with tile.TileContext(nc) as tc:
        with (
            tc.tile_pool(name="inp", bufs=1) as sbin,
            tc.tile_pool(name="outp", bufs=4) as sbout,
            tc.tile_pool(name="ps", bufs=2, space="PSUM") as ps,
        ):
            # warm the ACT exp table-set load (~2.7 us) under the input DMAs
            warm = sbout.tile([128, 8], F32, name="actwarm")
            nc.scalar.activation(warm[:], warm[:], mybir.ActivationFunctionType.Exp)

            # chunked input tiles: the first matmuls gate on the lhs chunk
            # plus one 2048-col rhs chunk instead of the whole 2.4 MiB input
            p_lhs = sbin.tile([K, RPC], F32R, name="pl")
            p_rhs = [sbin.tile([K, CBW], F32R, name=f"pr{h}") for h in range(NCH)]
            loop = tc.For_i(0, loop_r) if loop_r else contextlib.nullcontext()
            with loop:
                nc.sync.dma_start(p_lhs[:], p_d[:, :RPC])
                for h in range(NCH):
                    c0 = RPC + h * CBW
                    nc.sync.dma_start(p_rhs[h][:], p_d[:, c0:c0 + CBW])

                n_dma = 0
                for r in range(R_TILES):
                    lhs = p_lhs[:, r * 128:(r + 1) * 128]
                    for cb in range(C_BLOCKS):
                        acc = ps.tile([128, CBW], F32)
                        for j in range(CBW // MM_W):
                            seg = slice(j * MM_W, (j + 1) * MM_W)
                            nc.tensor.matmul(
                                acc[:, seg], lhs, p_rhs[cb][:, seg],
                                start=True, stop=True,
                            )
                        ot = sbout.tile([128, CBW], BF)
                        nc.scalar.activation(
                            ot[:], acc[:], mybir.ActivationFunctionType.Exp
                        )
                        # alternate between the two HWDGE rings (SP and ACT)
                        eng = nc.scalar if n_dma % 2 else nc.sync
                        eng.dma_start(
                            out_d[r * 128:(r + 1) * 128, cb * CBW:(cb + 1) * CBW],
                            ot[:],
                        )
                        n_dma += 1
                if tick_d is not None:
                    nc.sync.dma_start(tick_d[:, :], warm[:, :8])
    _split_excess_waits(nc)
    return nc


def prepare_in_maps(x, y):
    x = np.asarray(x, dtype=np.float32)
    y = np.asarray(y, dtype=np.float32)
    assert x.shape == (N_ROWS, D) and y.shape == (N_COLS, D)

    x_sq = (x.astype(np.float64) ** 2).sum(axis=1).astype(np.float32)
    y_sq = (y.astype(np.float64) ** 2).sum(axis=1).astype(np.float32)

    # rhs half is shared by all cores: [2y.T; -1; -ysq]
    rhs = np.concatenate(
        [2.0 * y.T, -np.ones((1, N_COLS), np.float32), -y_sq[None, :]], axis=0
    ).astype(np.float32)

    in_maps = []
    for c in range(N_CORES):
        rows = slice(c * RPC, (c + 1) * RPC)
        lhs = np.concatenate(
            [x.T[:, rows], x_sq[None, rows], np.ones((1, RPC), np.float32)], axis=0
        ).astype(np.float32)
        in_maps.append({"p": np.concatenate([lhs, rhs], axis=1)})
    return in_maps


def kernel(x, y):
    in_maps = prepare_in_maps(x, y)
    nc = build_nc()
    res = run_bass_kernel_spmd(nc, in_maps, core_ids=list(range(N_CORES)))
    out = np.concatenate(
        [np.asarray(res.results[c]["out"]) for c in range(N_CORES)], axis=0
    )
    return out.astype(np.float32)
